# revision 1
# baseline (speedup 1.0000x reference)
"""AgentAttention Trainium2 kernel (B=64, N=1024, C=512, M=16 agents) on 8 NeuronCores.

Data-parallel over batch: each core processes 8 batch elements. No collectives.

Math (per batch element, reference semantics):
    Q = x@Wq.T+bq ; K = x@Wk.T+bk ; V = x@Wv.T+bv
    A = group-mean of Q over 64-token groups          -> [16, C]
    S1 = softmax(Q A^T / sqrt(C), axis=agents)        -> [N, 16]
    S2 = softmax(A K^T / sqrt(C), axis=tokens)        -> [16, N]
    out = (S1 @ (S2 @ V)) @ Wo.T + bo

Algebraic restructuring used on device (exact in real arithmetic):
    - bv never materialized: softmax rows sum to 1 =>  out += (Wo@bv + bo) == b'
    - A uses group-SUM; the 1/64 is folded into the logit scale s = 1/(64*sqrt(C))
    - re-association: S1@((S2@x)@(Wv^T Wo^T)) replaces the O(N*C^2) V- and
      output-projections with agent-space (M=16) ops; Wvo^T = Wv^T@Wo^T is
      precomputed on host in float64.
    - stage-1 softmax normalizer r1 applied as a per-row scale on the final
      output tile; stage-2 normalizer r2 applied when evicting (S2@x) from PSUM.
All matmuls in bf16 with fp32 PSUM accumulation; XBAR DMA transposes for the
[16,*] -> [*,16] layout changes (no PE transposes).
"""

import sys
import os

if "/opt/trn_rl_repo" not in sys.path:
    sys.path.insert(0, "/opt/trn_rl_repo")

import numpy as np
import ml_dtypes

import concourse.bass as bass
import concourse.mybir as mybir
import concourse.tile as tile
from concourse import bacc
from concourse.bass import ts, ds
from concourse.bass_utils import run_bass_kernel_spmd
from concourse.masks import make_identity

BF16 = mybir.dt.bfloat16
F32 = mybir.dt.float32

N_CORES = 8
B = 64
B_LOC = B // N_CORES  # 8 batches per core
N = 1024              # tokens
C = 512               # channels
M = 16                # agents
G = N // M            # 64-token pooling groups
P = 128
ND = C // P           # 4 channel chunks
NN = N // P           # 8 token chunks of 128
NI = N // 512         # 2 token chunks of 512
SCALE = 1.0 / (G * np.sqrt(C))  # logit scale (1/64 pooling fold included)

# test harness may override (e.g. {"trace": True, "tmpdir": ...})
_RUN_KWARGS = {}
_LAST_RESULTS = None


def _build_program():
    nc = bacc.Bacc("TRN2", target_bir_lowering=False, debug=False,
                   num_devices=N_CORES)

    xT_d = nc.dram_tensor("xT", [B_LOC, C, N], BF16, kind="ExternalInput")
    xs_d = nc.dram_tensor("xsumT", [B_LOC, C, M], BF16, kind="ExternalInput")
    xn_d = nc.dram_tensor("xn", [B_LOC, N, C], BF16, kind="ExternalInput")
    wqT_d = nc.dram_tensor("wqT", [C, C], BF16, kind="ExternalInput")
    wqN_d = nc.dram_tensor("wqN", [C, C], BF16, kind="ExternalInput")
    wkN_d = nc.dram_tensor("wkN", [C, C], BF16, kind="ExternalInput")
    wvo_d = nc.dram_tensor("wvoT", [C, C], BF16, kind="ExternalInput")
    bq64_d = nc.dram_tensor("bq64", [C], F32, kind="ExternalInput")
    bqc_d = nc.dram_tensor("bqc", [C], BF16, kind="ExternalInput")
    bkc_d = nc.dram_tensor("bkc", [C], BF16, kind="ExternalInput")
    bp_d = nc.dram_tensor("bp", [P, C], BF16, kind="ExternalInput")
    out_d = nc.dram_tensor("out", [B_LOC, N, C], BF16, kind="ExternalOutput")

    with tile.TileContext(nc) as tc:
        with (
            tc.tile_pool(name="const", bufs=1) as const,
            tc.tile_pool(name="px", bufs=3) as px,
            tc.tile_pool(name="pxn", bufs=5) as pxn,
            tc.tile_pool(name="psmall", bufs=5) as psmall,
            tc.tile_pool(name="pout", bufs=6) as pout,
            tc.tile_pool(name="ps_mm", bufs=2, space="PSUM") as ps_mm,
            tc.tile_pool(name="ps_log", bufs=2, space="PSUM") as ps_log,
            tc.tile_pool(name="ps_s16", bufs=2, space="PSUM") as ps_s16,
            tc.tile_pool(name="ps_c", bufs=1, space="PSUM") as ps_c,
            tc.tile_pool(name="ps_tr", bufs=1, space="PSUM") as ps_tr,
        ):
            wqT_s = const.tile([P, ND, C], BF16)
            wqN_s = const.tile([P, ND, C], BF16)
            wkN_s = const.tile([P, ND, C], BF16)
            wvo_s = const.tile([P, ND, C], BF16)
            bq64_s = const.tile([P, ND], F32)
            bqc_s = const.tile([P, ND], BF16)
            bkc_s = const.tile([P, ND], BF16)
            bp_s = const.tile([P, C], BF16)
            xsall_s = const.tile([P, B_LOC, ND, M], BF16)
            nc.sync.dma_start(wqT_s[:], wqT_d.ap().rearrange("(o p) d -> p o d", p=P))
            nc.sync.dma_start(
                xsall_s[:], xs_d.ap().rearrange("b (o p) m -> p b o m", p=P))
            nc.sync.dma_start(bq64_s[:], bq64_d.ap().rearrange("(o p) -> p o", p=P))

            def load_late_consts():
                nc.sync.dma_start(wvo_s[:], wvo_d.ap().rearrange("(o p) d -> p o d", p=P))
                nc.sync.dma_start(bp_s[:], bp_d.ap())

            ident = const.tile([M, M], BF16)
            make_identity(nc, ident)

            st = [dict() for _ in range(B_LOC)]

            def load_x(b):
                s = st[b]
                s["xt"] = xt = px.tile([P, ND, N], BF16, tag="xt", name=f"xt_{b}")
                srcap = xT_d.ap()[b].rearrange("(o p) n -> p o n", p=P)
                if b == 0:
                    for c in range(ND):
                        nc.sync.dma_start(xt[:, c:c + 1], srcap[:, c:c + 1])
                else:
                    nc.sync.dma_start(xt[:], srcap)
                s["xn"] = xn = pxn.tile([P, NN, C], BF16, tag="xn", name=f"xn_{b}")
                nc.sync.dma_start(
                    xn[:], xn_d.ap()[b].rearrange("(o p) c -> p o c", p=P))

            def asum_mm(b):
                # Asum^T[d, m] = Wq^T.T @ xsum^T + 64*bq  (pooling folded through Wq)
                s = st[b]
                s["asum_b"] = ab = psmall.tile([P, ND, M], BF16, tag="asum_b", name=f"asum_b_{b}")
                for d in range(ND):
                    ps = ps_s16.tile([P, M], F32, tag="s16", name=f"s16_{b}")
                    for c in range(ND):
                        nc.tensor.matmul(
                            ps[:], wqT_s[:, c, ds(d * P, P)], xsall_s[:, b, c, :],
                            start=(c == 0), stop=(c == ND - 1))
                    nc.scalar.activation(
                        ab[:, d, :], ps[:],
                        mybir.ActivationFunctionType.Identity,
                        bias=bq64_s[:, d:d + 1])

            def awq_mm(b):
                # AWQ^T[c, m] = Wq.T @ Asum^T
                s = st[b]
                s["awqT"] = aw = psmall.tile([P, ND, M], BF16, tag="awqT", name=f"awqT_{b}")
                for c in range(ND):
                    ps = ps_s16.tile([P, M], F32, tag="s16", name=f"s16_{b}")
                    for d in range(ND):
                        nc.tensor.matmul(
                            ps[:], wqN_s[:, d, ds(c * P, P)], s["asum_b"][:, d, :],
                            start=(d == 0), stop=(d == ND - 1))
                    nc.scalar.activation(
                        aw[:, c, :], ps[:], mybir.ActivationFunctionType.Copy)

            def awk_mm(b):
                # AWK^T[c, m] = Wk.T @ Asum^T ; c1 = s*(Asum@bq) ; c2 = s*(Asum@bk)
                s = st[b]
                s["awkT"] = aw = psmall.tile([P, ND, M], BF16, tag="awkT", name=f"awkT_{b}")
                for c in range(ND):
                    ps = ps_s16.tile([P, M], F32, tag="s16", name=f"s16_{b}")
                    for d in range(ND):
                        nc.tensor.matmul(
                            ps[:], wkN_s[:, d, ds(c * P, P)], s["asum_b"][:, d, :],
                            start=(d == 0), stop=(d == ND - 1))
                    nc.scalar.activation(
                        aw[:, c, :], ps[:], mybir.ActivationFunctionType.Copy)
                for (col, key) in ((bqc_s, "c1s"), (bkc_s, "c2s")):
                    pc_ = ps_c.tile([M, 1], F32, tag="c", name=f"c_{b}")
                    for d in range(ND):
                        nc.tensor.matmul(
                            pc_[:], s["asum_b"][:, d, :], col[:, d:d + 1],
                            start=(d == 0), stop=(d == ND - 1))
                    s[key] = ct = psmall.tile([M, 1], F32, tag=key, name=f"{key}_{b}")
                    nc.scalar.activation(
                        ct[:], pc_[:], mybir.ActivationFunctionType.Copy,
                        scale=float(SCALE))

            def mid_l1(b):
                s = st[b]
                s["e1t"] = e1t = psmall.tile([M, N], BF16, tag="e1t", name=f"e1t_{b}")
                for ni in range(NI):
                    psl = ps_log.tile([M, 512], F32, tag="log", name=f"log_{b}")
                    for c in range(ND):
                        nc.tensor.matmul(
                            psl[:], s["awqT"][:, c, :], s["xt"][:, c, ts(ni, 512)],
                            start=(c == 0), stop=(c == ND - 1))
                    nc.scalar.activation(
                        e1t[:, ts(ni, 512)], psl[:],
                        mybir.ActivationFunctionType.Exp,
                        bias=s["c1s"][:], scale=float(SCALE))

            def mid_l2(b):
                s = st[b]
                s["e2"] = e2 = psmall.tile([M, N], BF16, tag="e2", name=f"e2_{b}")
                s["d2"] = d2 = psmall.tile([M, NI], F32, tag="d2", name=f"d2_{b}")
                for ni in range(NI):
                    psl = ps_log.tile([M, 512], F32, tag="log", name=f"log_{b}")
                    for c in range(ND):
                        nc.tensor.matmul(
                            psl[:], s["awkT"][:, c, :], s["xt"][:, c, ts(ni, 512)],
                            start=(c == 0), stop=(c == ND - 1))
                    nc.scalar.activation(
                        e2[:, ts(ni, 512)], psl[:],
                        mybir.ActivationFunctionType.Exp,
                        bias=s["c2s"][:], scale=float(SCALE),
                        accum_out=d2[:, ni:ni + 1])
                d2s = psmall.tile([M, 1], F32, tag="d2s", name=f"d2s_{b}")
                nc.vector.tensor_add(d2s[:], d2[:, 0:1], d2[:, 1:2])
                s["r2"] = r2 = psmall.tile([M, 1], F32, tag="r2", name=f"r2_{b}")
                nc.vector.reciprocal(r2[:], d2s[:])

            def mid_e2t(b):
                # e2t[p, o, m] = E2[m, o*128+p] via XBAR dma transpose
                s = st[b]
                s["e2t"] = e2t = psmall.tile([P, NN, M], BF16, tag="e2t", name=f"e2t_{b}")
                nc.sync.dma_start_transpose(e2t[:, 0:NN // 2], s["e2"][:, 0:512])
                nc.sync.dma_start_transpose(e2t[:, NN // 2:], s["e2"][:, 512:])

            def mid_r1(b):
                # r1[n] = sum_m E1^T[m, n] via XBAR transpose + free-dim reduce
                s = st[b]
                e1p = psmall.tile([P, NN, M], BF16, tag="e1p", name=f"e1p_{b}")
                nc.sync.dma_start_transpose(e1p[:], s["e1t"][:])
                r_s = psmall.tile([P, NN], F32, tag="r_s", name=f"r_s_{b}")
                nc.vector.reduce_sum(r_s[:], e1p[:], axis=mybir.AxisListType.X)
                s["r_inv"] = r_inv = psmall.tile([P, NN], F32, tag="r_inv", name=f"r_inv_{b}")
                nc.vector.reciprocal(r_inv[:], r_s[:])

            def mid_ex(b):
                # exr = diag(r2) * (E2 @ x)   [16, C]
                s = st[b]
                pse = ps_log.tile([M, 512], F32, tag="log", name=f"log_{b}")
                for n in range(NN):
                    nc.tensor.matmul(
                        pse[:], s["e2t"][:, n, :], s["xn"][:, n, :],
                        start=(n == 0), stop=(n == NN - 1))
                s["exr"] = exr = psmall.tile([M, C], BF16, tag="exr", name=f"exr_{b}")
                nc.scalar.activation(
                    exr[:], pse[:], mybir.ActivationFunctionType.Copy,
                    scale=s["r2"][:])

            def mid_exrT(b):
                # exrT[p, c, m] = exr[m, c*128+p] via PE transposes
                s = st[b]
                pst = ps_tr.tile([P, ND, M], BF16, tag="tr", name=f"tr_{b}")
                for c in range(ND):
                    nc.tensor.transpose(pst[:, c, :], s["exr"][:, ts(c, P)],
                                        ident[:])
                s["exrT"] = exrT = psmall.tile([P, ND, M], BF16, tag="exrT", name=f"exrT_{b}")
                nc.vector.tensor_copy(exrT[:], pst[:])

            def mid_afw(b):
                # afw = exr @ Wvo^T   [16, C]
                s = st[b]
                psa = ps_log.tile([M, 512], F32, tag="log", name=f"log_{b}")
                for c in range(ND):
                    nc.tensor.matmul(
                        psa[:], s["exrT"][:, c, :], wvo_s[:, c, :],
                        start=(c == 0), stop=(c == ND - 1))
                s["afw"] = afw = psmall.tile([M, C], BF16, tag="afw", name=f"afw_{b}")
                nc.scalar.activation(
                    afw[:], psa[:], mybir.ActivationFunctionType.Copy)

            def out_proj(b, n_range=None):
                # out chunk = r1 * (E1 @ afw) + b'
                s = st[b]
                for n in (n_range if n_range is not None else range(NN)):
                    ps = ps_mm.tile([P, 512], F32, tag="mm", name=f"mm_{b}")
                    nc.tensor.matmul(
                        ps[:], s["e1t"][:, ts(n, P)], s["afw"][:],
                        start=True, stop=True)
                    o_s = pout.tile([P, C], BF16, tag="o", name=f"o_{b}")
                    nc.vector.tensor_scalar_mul(
                        o_s[:], ps[:], s["r_inv"][:, n:n + 1])
                    nc.vector.tensor_add(o_s[:], o_s[:], bp_s[:])
                    nc.sync.dma_start(out_d.ap()[b][ts(n, P), :], o_s[:])

            # software pipeline, 3 batches in flight:
            #   step k: agent-products(k) | attention-middle(k-1) | output(k-2)
            nc.sync.dma_start(wqN_s[:], wqN_d.ap().rearrange("(o p) d -> p o d", p=P))
            nc.sync.dma_start(wkN_s[:], wkN_d.ap().rearrange("(o p) d -> p o d", p=P))
            nc.sync.dma_start(bqc_s[:], bqc_d.ap().rearrange("(o p) -> p o", p=P))
            nc.sync.dma_start(bkc_s[:], bkc_d.ap().rearrange("(o p) -> p o", p=P))
            load_x(0)
            load_late_consts()
            if B_LOC > 1:
                load_x(1)
            for k in range(B_LOC + 2):
                if 0 < k and k + 2 < B_LOC:
                    load_x(k + 2)
                if k == 0:
                    # batch 0's agent products fill the initial x-load window
                    asum_mm(0)
                    awq_mm(0)
                    awk_mm(0)
                if k < B_LOC:
                    # agent products for batch k+1 run a step early (they only
                    # need the resident pooled sums), giving the awqT/awkT ->
                    # L1/L2 hand-off a full step of slack
                    if 0 <= k - 2:
                        mid_ex(k - 2)
                    if k + 1 < B_LOC:
                        asum_mm(k + 1)
                    if 0 <= k - 2:
                        mid_exrT(k - 2)
                    if k + 1 < B_LOC:
                        awq_mm(k + 1)
                    if 0 <= k - 2:
                        mid_afw(k - 2)
                    if k + 1 < B_LOC:
                        awk_mm(k + 1)
                    if 0 <= k - 3:
                        out_proj(k - 3, range(0, 4))
                    mid_l2(k)
                    mid_e2t(k)
                    mid_l1(k)
                    mid_r1(k)
                    if 0 <= k - 3:
                        out_proj(k - 3, range(4, 8))
                elif k == B_LOC:
                    # drain step 1: finish batches B-3/B-2's chains with
                    # batch B-3's output as filler, then batch B-2's output
                    mid_ex(k - 2)
                    out_proj(k - 3, range(0, 4))
                    mid_exrT(k - 2)
                    mid_afw(k - 2)
                    mid_ex(k - 1)
                    out_proj(k - 3, range(4, 8))
                    mid_exrT(k - 1)
                    mid_afw(k - 1)
                    out_proj(k - 2)
                elif k == B_LOC + 1:
                    out_proj(k - 2)
                if k == 0:
                    load_x(2)

    nc.compile()
    return nc


def _prep_inputs(x, Wq, bq, Wk, bk, Wv, bv, Wo, bo):
    bf = ml_dtypes.bfloat16
    x32 = np.asarray(x, np.float32)
    xT = np.ascontiguousarray(x32.transpose(0, 2, 1)).astype(bf)
    xn = np.ascontiguousarray(x32).astype(bf)
    xsT = np.ascontiguousarray(
        x32.reshape(B, M, G, C).sum(axis=2).transpose(0, 2, 1)).astype(bf)
    Wo64 = np.asarray(Wo, np.float64)
    Wv64 = np.asarray(Wv, np.float64)
    shared = {
        "wqT": np.ascontiguousarray(np.asarray(Wq, np.float32).T).astype(bf),
        "wqN": np.ascontiguousarray(np.asarray(Wq, np.float32)).astype(bf),
        "wkN": np.ascontiguousarray(np.asarray(Wk, np.float32)).astype(bf),
        "wvoT": np.ascontiguousarray((Wo64 @ Wv64).T.astype(np.float32)).astype(bf),
        "bq64": 64.0 * np.asarray(bq, np.float32),
        "bqc": np.asarray(bq, np.float32).astype(bf),
        "bkc": np.asarray(bk, np.float32).astype(bf),
    }
    bprime = np.asarray(bo, np.float64) + Wo64 @ np.asarray(bv, np.float64)
    shared["bp"] = np.tile(bprime.astype(np.float32), (P, 1)).astype(bf)
    in_maps = []
    for c in range(N_CORES):
        m = dict(shared)
        m["xT"] = np.ascontiguousarray(xT[c * B_LOC:(c + 1) * B_LOC])
        m["xsumT"] = np.ascontiguousarray(xsT[c * B_LOC:(c + 1) * B_LOC])
        m["xn"] = np.ascontiguousarray(xn[c * B_LOC:(c + 1) * B_LOC])
        in_maps.append(m)
    return in_maps


def kernel(x, Wq, bq, Wk, bk, Wv, bv, Wo, bo):
    global _LAST_RESULTS
    nc = _build_program()
    in_maps = _prep_inputs(x, Wq, bq, Wk, bk, Wv, bv, Wo, bo)
    res = run_bass_kernel_spmd(nc, in_maps, list(range(N_CORES)), **_RUN_KWARGS)
    _LAST_RESULTS = res
    out = np.concatenate([res.results[i]["out"] for i in range(N_CORES)], axis=0)
    return out.astype(np.float32)



# revision 6
# speedup vs baseline: 1.0582x; 1.0582x over previous
"""AgentAttention Trainium2 kernel (B=64, N=1024, C=512, M=16 agents) on 8 NeuronCores.

Data-parallel over batch: each core processes 8 batch elements. No collectives.

Math (per batch element, reference semantics):
    Q = x@Wq.T+bq ; K = x@Wk.T+bk ; V = x@Wv.T+bv
    A = group-mean of Q over 64-token groups          -> [16, C]
    S1 = softmax(Q A^T / sqrt(C), axis=agents)        -> [N, 16]
    S2 = softmax(A K^T / sqrt(C), axis=tokens)        -> [16, N]
    out = (S1 @ (S2 @ V)) @ Wo.T + bo

Algebraic restructuring (exact in real arithmetic):
    - bv never materialized: softmax rows sum to 1 =>  out += (Wo@bv + bo) == b'
    - A uses group-SUM; the 1/64 is folded into the logit scale s = 1/(64*sqrt(C))
    - re-association: S1@((S2@x)@(Wv^T Wo^T)) replaces the O(N*C^2) V- and
      output-projections with agent-space (M=16) ops; Wvo^T = (Wo@Wv)^T is
      precomputed on host in float64.
    - the A@bk logit shift is constant along the stage-2 softmax axis and
      cancels; it is dropped entirely.
    - stage-1 softmax normalizer r1 applied as a per-row scale on the final
      output tile; stage-2 normalizer r2 applied when evicting (S2@x) from PSUM.

Perf structure (v2):
    - agent products (Asum, AWQ^T, AWK^T, c1) computed ONCE for all 8 local
      batches with the 8*16=128 (batch,agent) pairs as a full 128-wide matmul
      free/partition dim -- 48 full-width matmuls instead of 448 16-wide ones.
    - AWQ^T/AWK^T stored interleaved as one [128, 32] stationary per
      (c-chunk, batch): stage-1 and stage-2 logits come out of ONE x^T stream.
    - per batch ONE pair of XBAR transposes moves [32, 1024] E1/E2 rows into
      token-partition layout (feeds both the ex matmul and the r1 reduce).
    - all HBM<->SBUF transfers are host-permuted so every SBUF partition
      reads/writes a single contiguous 8KB block (large DMA packets).
"""

import sys

if "/opt/trn_rl_repo" not in sys.path:
    sys.path.insert(0, "/opt/trn_rl_repo")

import numpy as np
import ml_dtypes

import concourse.bass as bass
import concourse.mybir as mybir
import concourse.tile as tile
from concourse import bacc
from concourse.bass import ts, ds
from concourse.bass_utils import run_bass_kernel_spmd
from concourse.masks import make_identity

BF16 = mybir.dt.bfloat16
F32 = mybir.dt.float32

N_CORES = 8
B = 64
B_LOC = B // N_CORES  # 8 batches per core
N = 1024              # tokens
C = 512               # channels
M = 16                # agents
G = N // M            # 64-token pooling groups
P = 128
ND = C // P           # 4 channel chunks
NN = N // P           # 8 token chunks of 128
J = B_LOC * M         # 128 stacked (batch, agent) columns
SCALE = 1.0 / (G * np.sqrt(C))  # logit scale (1/64 pooling fold included)

# test harness may override (e.g. {"trace": True, "tmpdir": ...})
_RUN_KWARGS = {}
_LAST_RESULTS = None


def _build_program():
    nc = bacc.Bacc("TRN2", target_bir_lowering=False, debug=False,
                   num_devices=N_CORES)

    xt_d = nc.dram_tensor("xt", [B_LOC, P, ND, N], BF16, kind="ExternalInput")
    xn_d = nc.dram_tensor("xn", [B_LOC, P, NN, C], BF16, kind="ExternalInput")
    xs_d = nc.dram_tensor("xs", [P, ND, J], BF16, kind="ExternalInput")
    wqT_d = nc.dram_tensor("wqT", [P, ND, C], BF16, kind="ExternalInput")
    wqN_d = nc.dram_tensor("wqN", [P, ND, C], BF16, kind="ExternalInput")
    wkN_d = nc.dram_tensor("wkN", [P, ND, C], BF16, kind="ExternalInput")
    wvo_d = nc.dram_tensor("wvoT", [P, ND, C], BF16, kind="ExternalInput")
    bq64_d = nc.dram_tensor("bq64", [P, ND], F32, kind="ExternalInput")
    bp_d = nc.dram_tensor("bp", [P, C], BF16, kind="ExternalInput")
    c12b_d = nc.dram_tensor("c12b", [3 * M, B_LOC], F32, kind="ExternalInput")
    out_d = nc.dram_tensor("out", [B_LOC, P, NN, C], BF16, kind="ExternalOutput")

    with tile.TileContext(nc) as tc:
        with (
            tc.tile_pool(name="const", bufs=1) as const,
            tc.tile_pool(name="pxt", bufs=3) as pxt,
            tc.tile_pool(name="pxn", bufs=3) as pxn,
            tc.tile_pool(name="pe12", bufs=3) as pe12,
            tc.tile_pool(name="pe12p", bufs=3) as pe12p,
            tc.tile_pool(name="psmall", bufs=4) as psmall,
            tc.tile_pool(name="pout", bufs=2) as pout,
            tc.tile_pool(name="ps_big", bufs=3, space="PSUM") as ps_big,
            tc.tile_pool(name="ps_log", bufs=2, space="PSUM") as ps_log,
            tc.tile_pool(name="ps_se", bufs=2, space="PSUM") as ps_se,
        ):
            wqT_s = const.tile([P, ND, C], BF16)
            wqN_s = const.tile([P, ND, C], BF16)
            wkN_s = const.tile([P, ND, C], BF16)
            wvo_s = const.tile([P, ND, C], BF16)
            xs_s = const.tile([P, ND, J], BF16)
            bq64_s = const.tile([P, ND], F32)
            bp_s = const.tile([P, C], BF16)
            asum_s = const.tile([P, ND, J], BF16)
            awqk_s = const.tile([P, ND, B_LOC, 3 * M], BF16)
            c12b_s = const.tile([3 * M, B_LOC], F32)
            ident = const.tile([M, M], BF16)

            # first wave of const loads: everything the agent stage needs
            nc.sync.dma_start(wqT_s[:], wqT_d.ap())
            nc.sync.dma_start(xs_s[:], xs_d.ap())
            nc.sync.dma_start(bq64_s[:], bq64_d.ap())
            nc.sync.dma_start(wqN_s[:], wqN_d.ap())
            nc.sync.dma_start(wkN_s[:], wkN_d.ap())
            nc.sync.dma_start(c12b_s[:], c12b_d.ap())
            make_identity(nc, ident)
            nc.vector.memset(awqk_s[:, :, :, M:2 * M], 0.0)

            st = [dict() for _ in range(B_LOC)]

            def load_x(b):
                s = st[b]
                s["xt"] = xt = pxt.tile([P, ND, N], BF16, tag="xt", name=f"xt_{b}")
                nc.sync.dma_start(xt[:], xt_d.ap()[b])
                s["xn"] = xn = pxn.tile([P, NN, C], BF16, tag="xn", name=f"xn_{b}")
                nc.sync.dma_start(xn[:], xn_d.ap()[b])

            def agent_stage():
                # AsumT[d, j] = sum_c Wq^T[c, d] xsum^T[c, j] + 64 bq[d]
                for d in range(ND):
                    ps = ps_big.tile([P, B_LOC, M], F32, tag="mm", name="ag")
                    for c in range(ND):
                        nc.tensor.matmul(
                            ps[:], wqT_s[:, c, ds(d * P, P)], xs_s[:, c, :],
                            start=(c == 0), stop=(c == ND - 1))
                    nc.scalar.activation(
                        asum_s[:, d, :], ps[:],
                        mybir.ActivationFunctionType.Identity,
                        bias=bq64_s[:, d:d + 1])
                # AWQ^T[c, j] and AWK^T[c, j], interleaved per batch as the
                # fused [128, 32] logit stationary
                for (w_s, half) in ((wqN_s, 0), (wkN_s, 2)):
                    for c in range(ND):
                        ps = ps_big.tile([P, B_LOC, M], F32, tag="mm", name="ag")
                        for d in range(ND):
                            nc.tensor.matmul(
                                ps[:], w_s[:, d, ds(c * P, P)], asum_s[:, d, :],
                                start=(d == 0), stop=(d == ND - 1))
                        nc.scalar.activation(
                            awqk_s[:, c, :, half * M:(half + 1) * M], ps[:],
                            mybir.ActivationFunctionType.Copy)

            def l12(b):
                # E12^T[0:16, n] = exp(s*Q A^T + c1) ; [16:32, n] = exp(s*A K^T)
                s = st[b]
                s["e12t"] = e12t = pe12.tile([3 * M, N], BF16, tag="e12t",
                                             name=f"e12t_{b}")
                d2 = psmall.tile([3 * M, 2], F32, tag="d2", name=f"d2_{b}")
                for ni in range(2):
                    lg = ps_log.tile([3 * M, 512], F32, tag="log", name=f"log_{b}")
                    for c in range(ND):
                        nc.tensor.matmul(
                            lg[:], awqk_s[:, c, b, :], s["xt"][:, c, ts(ni, 512)],
                            start=(c == 0), stop=(c == ND - 1))
                    nc.scalar.activation(
                        e12t[:, ts(ni, 512)], lg[:],
                        mybir.ActivationFunctionType.Exp,
                        bias=c12b_s[:, b:b + 1], scale=float(SCALE),
                        accum_out=d2[:, ni:ni + 1])
                d2s = psmall.tile([M, 1], F32, tag="d2s", name=f"d2s_{b}")
                nc.vector.tensor_add(d2s[:], d2[2 * M:3 * M, 0:1], d2[2 * M:3 * M, 1:2])
                s["r2"] = r2 = psmall.tile([M, 1], F32, tag="r2", name=f"r2_{b}")
                nc.vector.reciprocal(r2[:], d2s[:])

            def tr(b):
                # e12p[p, o, i] = E12^T[i, o*128+p] via XBAR dma transpose
                s = st[b]
                s["e12p"] = e12p = pe12p.tile([P, NN, 2 * M], BF16, tag="e12p",
                                              name=f"e12p_{b}")
                e12t = s["e12t"]
                h = NN // 2
                nc.sync.dma_start_transpose(
                    e12p[:, 0:h, 0:M], e12t[0:M, 0:512])
                nc.sync.dma_start_transpose(
                    e12p[:, 0:h, M:2 * M], e12t[2 * M:3 * M, 0:512])
                nc.sync.dma_start_transpose(
                    e12p[:, h:NN, 0:M], e12t[0:M, 512:])
                nc.sync.dma_start_transpose(
                    e12p[:, h:NN, M:2 * M], e12t[2 * M:3 * M, 512:])

            def r1(b):
                # r1[n] = sum_m E1^T[m, n]; free-dim reduce in token-partition
                s = st[b]
                r_s = psmall.tile([P, NN], F32, tag="r_s", name=f"r_s_{b}")
                nc.vector.reduce_sum(r_s[:], s["e12p"][:, :, 0:M],
                                     axis=mybir.AxisListType.X)
                s["r_inv"] = r_inv = psmall.tile([P, NN], F32, tag="r_inv",
                                                 name=f"r_inv_{b}")
                nc.vector.reciprocal(r_inv[:], r_s[:])

            def ex(b):
                # exr = diag(r2) * (E2 @ x)   [16, C]
                s = st[b]
                pse = ps_se.tile([M, C], F32, tag="se", name=f"se_{b}")
                for n in range(NN):
                    nc.tensor.matmul(
                        pse[:], s["e12p"][:, n, M:2 * M], s["xn"][:, n, :],
                        start=(n == 0), stop=(n == NN - 1))
                s["exr"] = exr = psmall.tile([M, C], BF16, tag="exr",
                                             name=f"exr_{b}")
                nc.scalar.activation(
                    exr[:], pse[:], mybir.ActivationFunctionType.Copy,
                    scale=s["r2"][:])

            def exrT(b):
                # exrT[p, c, m] = exr[m, c*128+p] via PE transposes
                s = st[b]
                pst = ps_se.tile([P, ND, M], BF16, tag="se", name=f"tr_{b}")
                for c in range(ND):
                    nc.tensor.transpose(pst[:, c, :], s["exr"][:, ts(c, P)],
                                        ident[:])
                s["exrT"] = exrT_ = psmall.tile([P, ND, M], BF16, tag="exrT",
                                                name=f"exrT_{b}")
                nc.vector.tensor_copy(exrT_[:], pst[:])

            def afw(b):
                # afw = exr @ Wvo^T   [16, C]
                s = st[b]
                psa = ps_se.tile([M, C], F32, tag="se", name=f"afw_{b}")
                for c in range(ND):
                    nc.tensor.matmul(
                        psa[:], s["exrT"][:, c, :], wvo_s[:, c, :],
                        start=(c == 0), stop=(c == ND - 1))
                s["afw"] = afw_ = psmall.tile([M, C], BF16, tag="afw",
                                              name=f"afw_{b}")
                nc.scalar.activation(
                    afw_[:], psa[:], mybir.ActivationFunctionType.Copy)

            def outp(b):
                # out chunk = r1 * (E1 @ afw) + b'
                s = st[b]
                s["o"] = o_s = pout.tile([P, NN, C], BF16, tag="o", name=f"o_{b}")
                for n in range(NN):
                    po = ps_big.tile([P, C], F32, tag="mm", name=f"mm_{b}")
                    nc.tensor.matmul(
                        po[:], s["e12t"][0:M, ts(n, P)], s["afw"][:],
                        start=True, stop=True)
                    nc.scalar.activation(
                        o_s[:, n, :], po[:], mybir.ActivationFunctionType.Copy,
                        scale=s["r_inv"][:, n:n + 1])
                    nc.vector.tensor_add(o_s[:, n, :], o_s[:, n, :], bp_s[:])

            def store(b):
                nc.sync.dma_start(out_d.ap()[b], st[b]["o"][:])

            # prologue: agent products overlap the first x loads
            load_x(0)
            agent_stage()
            load_x(1)
            nc.sync.dma_start(wvo_s[:], wvo_d.ap())
            nc.sync.dma_start(bp_s[:], bp_d.ap())

            for b in range(B_LOC):
                l12(b)
                tr(b)
                if b + 2 < B_LOC:
                    load_x(b + 2)
                r1(b)
                if b >= 1:
                    ex(b - 1)
                    exrT(b - 1)
                    afw(b - 1)
                    outp(b - 1)
                    store(b - 1)
            ex(B_LOC - 1)
            exrT(B_LOC - 1)
            afw(B_LOC - 1)
            outp(B_LOC - 1)
            store(B_LOC - 1)

    nc.compile()
    return nc


def _prep_inputs(x, Wq, bq, Wk, bk, Wv, bv, Wo, bo):
    bf = ml_dtypes.bfloat16
    x32 = np.asarray(x, np.float32)
    # xt[b, p, o, n] = x[b, n, o*128+p]  (contiguous 8KB per partition)
    xt = np.ascontiguousarray(
        x32.transpose(0, 2, 1).reshape(B, ND, P, N).transpose(0, 2, 1, 3)
    ).astype(bf)
    # xn[b, p, o, c] = x[b, o*128+p, c]
    xn = np.ascontiguousarray(
        x32.reshape(B, NN, P, C).transpose(0, 2, 1, 3)).astype(bf)
    # pooled sums, all local batches stacked: xs[p, o, b*16+m]
    xsum = x32.reshape(B, M, G, C).sum(axis=2)  # [B, M, C]
    Wo64 = np.asarray(Wo, np.float64)
    Wv64 = np.asarray(Wv, np.float64)

    def wtile(w):  # [C, C] -> [P, ND, C] with w[p, o, d] = W[o*128+p, d]
        return np.ascontiguousarray(
            np.asarray(w, np.float32).reshape(ND, P, C).transpose(1, 0, 2)
        ).astype(bf)

    shared = {
        "wqT": wtile(np.asarray(Wq, np.float32).T),
        "wqN": wtile(np.asarray(Wq, np.float32)),
        "wkN": wtile(np.asarray(Wk, np.float32)),
        "wvoT": wtile((Wo64 @ Wv64).T.astype(np.float32)),
        "bq64": np.ascontiguousarray(
            (64.0 * np.asarray(bq, np.float32)).reshape(ND, P).T),
    }
    bprime = np.asarray(bo, np.float64) + Wo64 @ np.asarray(bv, np.float64)
    shared["bp"] = np.tile(bprime.astype(np.float32), (P, 1)).astype(bf)
    in_maps = []
    for ci in range(N_CORES):
        m = dict(shared)
        m["xt"] = np.ascontiguousarray(xt[ci * B_LOC:(ci + 1) * B_LOC])
        m["xn"] = np.ascontiguousarray(xn[ci * B_LOC:(ci + 1) * B_LOC])
        xs_c = xsum[ci * B_LOC:(ci + 1) * B_LOC]  # [B_LOC, M, C]
        xs_j = xs_c.transpose(2, 0, 1).reshape(C, J)  # [C, J], j = b*16+m
        m["xs"] = np.ascontiguousarray(
            xs_j.reshape(ND, P, J).transpose(1, 0, 2)).astype(bf)
        # stage-1 logit bias c1[b, m] = SCALE * (Asum_b,m . bq), exact on host
        asum64 = xs_c.astype(np.float64) @ np.asarray(Wq, np.float64).T \
            + 64.0 * np.asarray(bq, np.float64)
        c1 = SCALE * (asum64 @ np.asarray(bq, np.float64))  # [B_LOC, M]
        c12b = np.zeros((3 * M, B_LOC), np.float32)
        c12b[0:M, :] = c1.T.astype(np.float32)
        m["c12b"] = c12b
        in_maps.append(m)
    return in_maps


def _unpermute_out(res):
    # out_d[b, p, o, c] = out[b, o*128+p, c]
    outs = []
    for ci in range(N_CORES):
        o = np.asarray(res.results[ci]["out"], np.float32)  # [B_LOC, P, NN, C]
        outs.append(o.transpose(0, 2, 1, 3).reshape(B_LOC, N, C))
    return np.concatenate(outs, axis=0)


def kernel(x, Wq, bq, Wk, bk, Wv, bv, Wo, bo):
    global _LAST_RESULTS
    nc = _build_program()
    in_maps = _prep_inputs(x, Wq, bq, Wk, bk, Wv, bv, Wo, bo)
    res = run_bass_kernel_spmd(nc, in_maps, list(range(N_CORES)), **_RUN_KWARGS)
    _LAST_RESULTS = res
    return _unpermute_out(res)


# revision 7
# speedup vs baseline: 1.1745x; 1.1099x over previous
"""AgentAttention Trainium2 kernel (B=64, N=1024, C=512, M=16 agents) on 8 NeuronCores.

Data-parallel over batch: each core processes 8 batch elements. No collectives.

Math (per batch element, reference semantics):
    Q = x@Wq.T+bq ; K = x@Wk.T+bk ; V = x@Wv.T+bv
    A = group-mean of Q over 64-token groups          -> [16, C]
    S1 = softmax(Q A^T / sqrt(C), axis=agents)        -> [N, 16]
    S2 = softmax(A K^T / sqrt(C), axis=tokens)        -> [16, N]
    out = (S1 @ (S2 @ V)) @ Wo.T + bo

Algebraic restructuring (exact in real arithmetic):
    - bv never materialized: softmax rows sum to 1 =>  out += (Wo@bv + bo) == b'
    - A uses group-SUM; the 1/64 is folded into the logit scale s = 1/(64*sqrt(C))
    - re-association: S1@((S2@x)@(Wv^T Wo^T)) replaces the O(N*C^2) V- and
      output-projections with agent-space (M=16) ops; Wvo^T = (Wo@Wv)^T is
      precomputed on host in float64.
    - the A@bk logit shift is constant along the stage-2 softmax axis and
      cancels; it is dropped entirely.
    - stage-1 softmax normalizer r1 applied as a per-row scale on the final
      output tile; stage-2 normalizer r2 applied when evicting (S2@x) from PSUM.

Perf structure (v2):
    - agent products (Asum, AWQ^T, AWK^T, c1) computed ONCE for all 8 local
      batches with the 8*16=128 (batch,agent) pairs as a full 128-wide matmul
      free/partition dim -- 48 full-width matmuls instead of 448 16-wide ones.
    - AWQ^T/AWK^T stored interleaved as one [128, 32] stationary per
      (c-chunk, batch): stage-1 and stage-2 logits come out of ONE x^T stream.
    - per batch ONE pair of XBAR transposes moves [32, 1024] E1/E2 rows into
      token-partition layout (feeds both the ex matmul and the r1 reduce).
    - all HBM<->SBUF transfers are host-permuted so every SBUF partition
      reads/writes a single contiguous 8KB block (large DMA packets).
"""

import sys

if "/opt/trn_rl_repo" not in sys.path:
    sys.path.insert(0, "/opt/trn_rl_repo")

import numpy as np
import ml_dtypes

import concourse.bass as bass
import concourse.mybir as mybir
import concourse.tile as tile
from concourse import bacc
from concourse.bass import ts, ds
from concourse.bass_utils import run_bass_kernel_spmd
from concourse.masks import make_identity

BF16 = mybir.dt.bfloat16
F32 = mybir.dt.float32

N_CORES = 8
B = 64
B_LOC = B // N_CORES  # 8 batches per core
N = 1024              # tokens
C = 512               # channels
M = 16                # agents
G = N // M            # 64-token pooling groups
P = 128
ND = C // P           # 4 channel chunks
NN = N // P           # 8 token chunks of 128
J = B_LOC * M         # 128 stacked (batch, agent) columns
SCALE = 1.0 / (G * np.sqrt(C))  # logit scale (1/64 pooling fold included)

# test harness may override (e.g. {"trace": True, "tmpdir": ...})
_RUN_KWARGS = {}
_LAST_RESULTS = None


def _build_program():
    nc = bacc.Bacc("TRN2", target_bir_lowering=False, debug=False,
                   num_devices=N_CORES)

    xt_d = nc.dram_tensor("xt", [B_LOC, P, ND, N], BF16, kind="ExternalInput")
    xn_d = nc.dram_tensor("xn", [B_LOC, P, NN, C], BF16, kind="ExternalInput")
    xs_d = nc.dram_tensor("xs", [P, ND, J], BF16, kind="ExternalInput")
    wqT_d = nc.dram_tensor("wqT", [P, ND, C], BF16, kind="ExternalInput")
    wqN_d = nc.dram_tensor("wqN", [P, ND, C], BF16, kind="ExternalInput")
    wkN_d = nc.dram_tensor("wkN", [P, ND, C], BF16, kind="ExternalInput")
    wvo_d = nc.dram_tensor("wvoT", [P, ND, C], BF16, kind="ExternalInput")
    bq64_d = nc.dram_tensor("bq64", [P, ND], F32, kind="ExternalInput")
    bp_d = nc.dram_tensor("bp", [P, C], BF16, kind="ExternalInput")
    c12b_d = nc.dram_tensor("c12b", [3 * M, B_LOC], F32, kind="ExternalInput")
    out_d = nc.dram_tensor("out", [B_LOC, P, NN, C], BF16, kind="ExternalOutput")

    with tile.TileContext(nc) as tc:
        with (
            tc.tile_pool(name="const", bufs=1) as const,
            tc.tile_pool(name="pxt", bufs=4) as pxt,
            tc.tile_pool(name="pxn", bufs=4) as pxn,
            tc.tile_pool(name="pe12", bufs=3) as pe12,
            tc.tile_pool(name="pe12p", bufs=3) as pe12p,
            tc.tile_pool(name="psmall", bufs=4) as psmall,
            tc.tile_pool(name="pout", bufs=3) as pout,
            tc.tile_pool(name="ps_big", bufs=3, space="PSUM") as ps_big,
            tc.tile_pool(name="ps_log", bufs=2, space="PSUM") as ps_log,
            tc.tile_pool(name="ps_se", bufs=2, space="PSUM") as ps_se,
        ):
            wqT_s = const.tile([P, ND, C], BF16)
            wqN_s = const.tile([P, ND, C], BF16)
            wkN_s = const.tile([P, ND, C], BF16)
            wvo_s = const.tile([P, ND, C], BF16)
            xs_s = const.tile([P, ND, J], BF16)
            bq64_s = const.tile([P, ND], F32)
            bp_s = const.tile([P, C], BF16)
            asum_s = const.tile([P, ND, J], BF16)
            awqk_s = const.tile([P, ND, B_LOC, 3 * M], BF16)
            c12b_s = const.tile([3 * M, B_LOC], F32)
            ident = const.tile([M, M], BF16)

            # first wave of const loads: everything the agent stage needs
            nc.sync.dma_start(wqT_s[:], wqT_d.ap())
            nc.sync.dma_start(xs_s[:], xs_d.ap())
            nc.sync.dma_start(bq64_s[:], bq64_d.ap())
            make_identity(nc, ident)
            nc.vector.memset(awqk_s[:, :, :, M:2 * M], 0.0)

            st = [dict() for _ in range(B_LOC)]

            def load_x(b):
                s = st[b]
                s["xt"] = xt = pxt.tile([P, ND, N], BF16, tag="xt", name=f"xt_{b}")
                nc.sync.dma_start(xt[:], xt_d.ap()[b])
                s["xn"] = xn = pxn.tile([P, NN, C], BF16, tag="xn", name=f"xn_{b}")
                nc.sync.dma_start(xn[:], xn_d.ap()[b])

            def agent_stage():
                # AsumT[d, j] = sum_c Wq^T[c, d] xsum^T[c, j] + 64 bq[d]
                for d in range(ND):
                    ps = ps_big.tile([P, B_LOC, M], F32, tag="mm", name="ag")
                    for c in range(ND):
                        nc.tensor.matmul(
                            ps[:], wqT_s[:, c, ds(d * P, P)], xs_s[:, c, :],
                            start=(c == 0), stop=(c == ND - 1))
                    nc.scalar.activation(
                        asum_s[:, d, :], ps[:],
                        mybir.ActivationFunctionType.Identity,
                        bias=bq64_s[:, d:d + 1])
                # AWQ^T[c, j] and AWK^T[c, j], interleaved per batch as the
                # fused [128, 32] logit stationary
                for (w_s, half) in ((wqN_s, 0), (wkN_s, 2)):
                    for c in range(ND):
                        ps = ps_big.tile([P, B_LOC, M], F32, tag="mm", name="ag")
                        for d in range(ND):
                            nc.tensor.matmul(
                                ps[:], w_s[:, d, ds(c * P, P)], asum_s[:, d, :],
                                start=(d == 0), stop=(d == ND - 1))
                        nc.scalar.activation(
                            awqk_s[:, c, :, half * M:(half + 1) * M], ps[:],
                            mybir.ActivationFunctionType.Copy)

            def l12(b):
                # E12^T[0:16, n] = exp(s*Q A^T + c1) ; [16:32, n] = exp(s*A K^T)
                s = st[b]
                s["e12t"] = e12t = pe12.tile([3 * M, N], BF16, tag="e12t",
                                             name=f"e12t_{b}")
                d2 = psmall.tile([3 * M, 2], F32, tag="d2", name=f"d2_{b}")
                for ni in range(2):
                    lg = ps_log.tile([3 * M, 512], F32, tag="log", name=f"log_{b}")
                    for c in range(ND):
                        nc.tensor.matmul(
                            lg[:], awqk_s[:, c, b, :], s["xt"][:, c, ts(ni, 512)],
                            start=(c == 0), stop=(c == ND - 1))
                    nc.scalar.activation(
                        e12t[:, ts(ni, 512)], lg[:],
                        mybir.ActivationFunctionType.Exp,
                        bias=c12b_s[:, b:b + 1], scale=float(SCALE),
                        accum_out=d2[:, ni:ni + 1])
                d2s = psmall.tile([M, 1], F32, tag="d2s", name=f"d2s_{b}")
                nc.vector.tensor_add(d2s[:], d2[2 * M:3 * M, 0:1], d2[2 * M:3 * M, 1:2])
                s["r2"] = r2 = psmall.tile([M, 1], F32, tag="r2", name=f"r2_{b}")
                nc.vector.reciprocal(r2[:], d2s[:])

            def tr(b):
                # e12p[p, o, i] = E12^T[i, o*128+p] via XBAR dma transpose
                s = st[b]
                s["e12p"] = e12p = pe12p.tile([P, NN, 3 * M], BF16, tag="e12p",
                                              name=f"e12p_{b}")
                e12t = s["e12t"]
                h = NN // 2
                nc.sync.dma_start_transpose(e12p[:, 0:h, :], e12t[:, 0:512])
                nc.sync.dma_start_transpose(e12p[:, h:NN, :], e12t[:, 512:])

            def r1(b):
                # r1[n] = sum_m E1^T[m, n]; free-dim reduce in token-partition
                s = st[b]
                r_s = psmall.tile([P, NN], F32, tag="r_s", name=f"r_s_{b}")
                nc.vector.reduce_sum(r_s[:], s["e12p"][:, :, 0:M],
                                     axis=mybir.AxisListType.X)
                s["r_inv"] = r_inv = psmall.tile([P, NN], F32, tag="r_inv",
                                                 name=f"r_inv_{b}")
                nc.vector.reciprocal(r_inv[:], r_s[:])

            def ex(b):
                # exr = diag(r2) * (E2 @ x)   [16, C]
                s = st[b]
                pse = ps_se.tile([M, C], F32, tag="se", name=f"se_{b}")
                for n in range(NN):
                    nc.tensor.matmul(
                        pse[:], s["e12p"][:, n, 2 * M:3 * M], s["xn"][:, n, :],
                        start=(n == 0), stop=(n == NN - 1))
                s["exr"] = exr = psmall.tile([M, C], BF16, tag="exr",
                                             name=f"exr_{b}")
                nc.scalar.activation(
                    exr[:], pse[:], mybir.ActivationFunctionType.Copy,
                    scale=s["r2"][:])

            def exrT(b):
                # exrT[p, c, m] = exr[m, c*128+p] via PE transposes
                s = st[b]
                pst = ps_se.tile([P, ND, M], BF16, tag="se", name=f"tr_{b}")
                for c in range(ND):
                    nc.tensor.transpose(pst[:, c, :], s["exr"][:, ts(c, P)],
                                        ident[:])
                s["exrT"] = exrT_ = psmall.tile([P, ND, M], BF16, tag="exrT",
                                                name=f"exrT_{b}")
                nc.vector.tensor_copy(exrT_[:], pst[:])

            def afw(b):
                # afw = exr @ Wvo^T   [16, C]
                s = st[b]
                psa = ps_se.tile([M, C], F32, tag="se", name=f"afw_{b}")
                for c in range(ND):
                    nc.tensor.matmul(
                        psa[:], s["exrT"][:, c, :], wvo_s[:, c, :],
                        start=(c == 0), stop=(c == ND - 1))
                s["afw"] = afw_ = psmall.tile([M, C], BF16, tag="afw",
                                              name=f"afw_{b}")
                nc.scalar.activation(
                    afw_[:], psa[:], mybir.ActivationFunctionType.Copy)
                nc.vector.tensor_add(afw_[:], afw_[:], bp_s[0:M, :])

            def outp(b):
                # out chunk = r1 * (E1 @ afw) + b'
                s = st[b]
                s["o"] = o_s = pout.tile([P, NN, C], BF16, tag="o", name=f"o_{b}")
                for n in range(NN):
                    po = ps_big.tile([P, C], F32, tag="mm", name=f"mm_{b}")
                    nc.tensor.matmul(
                        po[:], s["e12t"][0:M, ts(n, P)], s["afw"][:],
                        start=True, stop=True)
                    if n % 2 == 0:
                        nc.scalar.activation(
                            o_s[:, n, :], po[:],
                            mybir.ActivationFunctionType.Copy,
                            scale=s["r_inv"][:, n:n + 1])
                    else:
                        nc.vector.tensor_scalar_mul(
                            o_s[:, n, :], po[:], s["r_inv"][:, n:n + 1])

            def store(b):
                nc.sync.dma_start(out_d.ap()[b], st[b]["o"][:])

            # prologue: agent products overlap the first x loads
            load_x(0)
            nc.sync.dma_start(wqN_s[:], wqN_d.ap())
            nc.sync.dma_start(wkN_s[:], wkN_d.ap())
            nc.sync.dma_start(c12b_s[:], c12b_d.ap())
            agent_stage()
            load_x(1)
            nc.sync.dma_start(wvo_s[:], wvo_d.ap())
            nc.sync.dma_start(bp_s[:], bp_d.ap())
            load_x(2)

            for b in range(B_LOC):
                l12(b)
                tr(b)
                if b + 3 < B_LOC:
                    load_x(b + 3)
                r1(b)
                if b >= 1:
                    ex(b - 1)
                    exrT(b - 1)
                    afw(b - 1)
                    outp(b - 1)
                    store(b - 1)
            ex(B_LOC - 1)
            exrT(B_LOC - 1)
            afw(B_LOC - 1)
            outp(B_LOC - 1)
            store(B_LOC - 1)

    nc.compile()
    return nc


def _prep_inputs(x, Wq, bq, Wk, bk, Wv, bv, Wo, bo):
    bf = ml_dtypes.bfloat16
    x32 = np.asarray(x, np.float32)
    # xt[b, p, o, n] = x[b, n, o*128+p]  (contiguous 8KB per partition)
    xt = np.ascontiguousarray(
        x32.transpose(0, 2, 1).reshape(B, ND, P, N).transpose(0, 2, 1, 3)
    ).astype(bf)
    # xn[b, p, o, c] = x[b, o*128+p, c]
    xn = np.ascontiguousarray(
        x32.reshape(B, NN, P, C).transpose(0, 2, 1, 3)).astype(bf)
    # pooled sums, all local batches stacked: xs[p, o, b*16+m]
    xsum = x32.reshape(B, M, G, C).sum(axis=2)  # [B, M, C]
    Wo64 = np.asarray(Wo, np.float64)
    Wv64 = np.asarray(Wv, np.float64)

    def wtile(w):  # [C, C] -> [P, ND, C] with w[p, o, d] = W[o*128+p, d]
        return np.ascontiguousarray(
            np.asarray(w, np.float32).reshape(ND, P, C).transpose(1, 0, 2)
        ).astype(bf)

    shared = {
        "wqT": wtile(np.asarray(Wq, np.float32).T),
        "wqN": wtile(np.asarray(Wq, np.float32)),
        "wkN": wtile(np.asarray(Wk, np.float32)),
        "wvoT": wtile((Wo64 @ Wv64).T.astype(np.float32)),
        "bq64": np.ascontiguousarray(
            (64.0 * np.asarray(bq, np.float32)).reshape(ND, P).T),
    }
    bprime = np.asarray(bo, np.float64) + Wo64 @ np.asarray(bv, np.float64)
    shared["bp"] = np.tile(bprime.astype(np.float32), (P, 1)).astype(bf)
    in_maps = []
    for ci in range(N_CORES):
        m = dict(shared)
        m["xt"] = np.ascontiguousarray(xt[ci * B_LOC:(ci + 1) * B_LOC])
        m["xn"] = np.ascontiguousarray(xn[ci * B_LOC:(ci + 1) * B_LOC])
        xs_c = xsum[ci * B_LOC:(ci + 1) * B_LOC]  # [B_LOC, M, C]
        xs_j = xs_c.transpose(2, 0, 1).reshape(C, J)  # [C, J], j = b*16+m
        m["xs"] = np.ascontiguousarray(
            xs_j.reshape(ND, P, J).transpose(1, 0, 2)).astype(bf)
        # stage-1 logit bias c1[b, m] = SCALE * (Asum_b,m . bq), exact on host
        asum64 = xs_c.astype(np.float64) @ np.asarray(Wq, np.float64).T \
            + 64.0 * np.asarray(bq, np.float64)
        c1 = SCALE * (asum64 @ np.asarray(bq, np.float64))  # [B_LOC, M]
        c12b = np.zeros((3 * M, B_LOC), np.float32)
        c12b[0:M, :] = c1.T.astype(np.float32)
        m["c12b"] = c12b
        in_maps.append(m)
    return in_maps


def _unpermute_out(res):
    # out_d[b, p, o, c] = out[b, o*128+p, c]
    outs = []
    for ci in range(N_CORES):
        o = np.asarray(res.results[ci]["out"], np.float32)  # [B_LOC, P, NN, C]
        outs.append(o.transpose(0, 2, 1, 3).reshape(B_LOC, N, C))
    return np.concatenate(outs, axis=0)


def kernel(x, Wq, bq, Wk, bk, Wv, bv, Wo, bo):
    global _LAST_RESULTS
    nc = _build_program()
    in_maps = _prep_inputs(x, Wq, bq, Wk, bk, Wv, bv, Wo, bo)
    res = run_bass_kernel_spmd(nc, in_maps, list(range(N_CORES)), **_RUN_KWARGS)
    _LAST_RESULTS = res
    return _unpermute_out(res)


# revision 8
# speedup vs baseline: 1.2395x; 1.0554x over previous
"""AgentAttention Trainium2 kernel (B=64, N=1024, C=512, M=16 agents) on 8 NeuronCores.

Data-parallel over batch: each core processes 8 batch elements. No collectives.

Math (per batch element, reference semantics):
    Q = x@Wq.T+bq ; K = x@Wk.T+bk ; V = x@Wv.T+bv
    A = group-mean of Q over 64-token groups          -> [16, C]
    S1 = softmax(Q A^T / sqrt(C), axis=agents)        -> [N, 16]
    S2 = softmax(A K^T / sqrt(C), axis=tokens)        -> [16, N]
    out = (S1 @ (S2 @ V)) @ Wo.T + bo

Algebraic restructuring (exact in real arithmetic):
    - bv never materialized: softmax rows sum to 1 =>  out += (Wo@bv + bo) == b'
    - A uses group-SUM; the 1/64 is folded into the logit scale s = 1/(64*sqrt(C))
    - re-association: S1@((S2@x)@(Wv^T Wo^T)) replaces the O(N*C^2) V- and
      output-projections with agent-space (M=16) ops; Wvo^T = (Wo@Wv)^T is
      precomputed on host in float64.
    - the A@bk logit shift is constant along the stage-2 softmax axis and
      cancels; it is dropped entirely.
    - stage-1 softmax normalizer r1 applied as a per-row scale on the final
      output tile; stage-2 normalizer r2 applied when evicting (S2@x) from PSUM.

Perf structure (v2):
    - agent products (Asum, AWQ^T, AWK^T, c1) computed ONCE for all 8 local
      batches with the 8*16=128 (batch,agent) pairs as a full 128-wide matmul
      free/partition dim -- 48 full-width matmuls instead of 448 16-wide ones.
    - AWQ^T/AWK^T stored interleaved as one [128, 32] stationary per
      (c-chunk, batch): stage-1 and stage-2 logits come out of ONE x^T stream.
    - per batch ONE pair of XBAR transposes moves [32, 1024] E1/E2 rows into
      token-partition layout (feeds both the ex matmul and the r1 reduce).
    - all HBM<->SBUF transfers are host-permuted so every SBUF partition
      reads/writes a single contiguous 8KB block (large DMA packets).
"""

import sys

if "/opt/trn_rl_repo" not in sys.path:
    sys.path.insert(0, "/opt/trn_rl_repo")

import numpy as np
import ml_dtypes

import concourse.bass as bass
import concourse.mybir as mybir
import concourse.tile as tile
from concourse import bacc
from concourse.bass import ts, ds
from concourse.bass_utils import run_bass_kernel_spmd
from concourse.masks import make_identity

BF16 = mybir.dt.bfloat16
F32 = mybir.dt.float32

N_CORES = 8
B = 64
B_LOC = B // N_CORES  # 8 batches per core
N = 1024              # tokens
C = 512               # channels
M = 16                # agents
G = N // M            # 64-token pooling groups
P = 128
ND = C // P           # 4 channel chunks
NN = N // P           # 8 token chunks of 128
J = B_LOC * M         # 128 stacked (batch, agent) columns
SCALE = 1.0 / (G * np.sqrt(C))  # logit scale (1/64 pooling fold included)

# test harness may override (e.g. {"trace": True, "tmpdir": ...})
_RUN_KWARGS = {}
_LAST_RESULTS = None


def _build_program():
    nc = bacc.Bacc("TRN2", target_bir_lowering=False, debug=False,
                   num_devices=N_CORES)

    xt_d = nc.dram_tensor("xt", [B_LOC, P, ND, N], BF16, kind="ExternalInput")
    xn_d = nc.dram_tensor("xn", [B_LOC, P, NN, C], BF16, kind="ExternalInput")
    xs_d = nc.dram_tensor("xs", [P, ND, J], BF16, kind="ExternalInput")
    wqT_d = nc.dram_tensor("wqT", [P, ND, C], BF16, kind="ExternalInput")
    wqN_d = nc.dram_tensor("wqN", [P, ND, C], BF16, kind="ExternalInput")
    wkN_d = nc.dram_tensor("wkN", [P, ND, C], BF16, kind="ExternalInput")
    wvo_d = nc.dram_tensor("wvoT", [P, ND, C], BF16, kind="ExternalInput")
    bq64_d = nc.dram_tensor("bq64", [P, ND], F32, kind="ExternalInput")
    bp_d = nc.dram_tensor("bp", [P, C], BF16, kind="ExternalInput")
    c12b_d = nc.dram_tensor("c12b", [3 * M, B_LOC], F32, kind="ExternalInput")
    out_d = nc.dram_tensor("out", [B_LOC, P, NN, C], BF16, kind="ExternalOutput")

    with tile.TileContext(nc) as tc:
        with (
            tc.tile_pool(name="const", bufs=1) as const,
            tc.tile_pool(name="pxt", bufs=4) as pxt,
            tc.tile_pool(name="pxn", bufs=4) as pxn,
            tc.tile_pool(name="pe12", bufs=3) as pe12,
            tc.tile_pool(name="pe12p", bufs=3) as pe12p,
            tc.tile_pool(name="psmall", bufs=4) as psmall,
            tc.tile_pool(name="pout", bufs=3) as pout,
            tc.tile_pool(name="ps_big", bufs=3, space="PSUM") as ps_big,
            tc.tile_pool(name="ps_log", bufs=2, space="PSUM") as ps_log,
            tc.tile_pool(name="ps_se", bufs=2, space="PSUM") as ps_se,
        ):
            wqT_s = const.tile([P, ND, C], BF16)
            wqN_s = const.tile([P, ND, C], BF16)
            wkN_s = const.tile([P, ND, C], BF16)
            wvo_s = const.tile([P, ND, C], BF16)
            xs_s = const.tile([P, ND, J], BF16)
            bq64_s = const.tile([P, ND], F32)
            bp_s = const.tile([P, C], BF16)
            asum_s = const.tile([P, ND, J], BF16)
            awqk_s = const.tile([P, ND, B_LOC, 3 * M], BF16)
            c12b_s = const.tile([3 * M, B_LOC], F32)
            ident = const.tile([M, M], BF16)

            # first wave of const loads: everything the agent stage needs
            nc.sync.dma_start(wqT_s[:], wqT_d.ap())
            nc.sync.dma_start(xs_s[:], xs_d.ap())
            nc.sync.dma_start(bq64_s[:], bq64_d.ap())
            make_identity(nc, ident)
            nc.vector.memset(awqk_s[:, :, :, M:2 * M], 0.0)

            st = [dict() for _ in range(B_LOC)]

            def load_x(b):
                s = st[b]
                s["xt"] = xt = pxt.tile([P, ND, N], BF16, tag="xt", name=f"xt_{b}")
                nc.sync.dma_start(xt[:], xt_d.ap()[b])
                s["xn"] = xn = pxn.tile([P, NN, C], BF16, tag="xn", name=f"xn_{b}")
                nc.sync.dma_start(xn[:], xn_d.ap()[b])

            def agent_stage():
                # AsumT[d, j] = sum_c Wq^T[c, d] xsum^T[c, j] + 64 bq[d]
                for d in range(ND):
                    ps = ps_big.tile([P, B_LOC, M], F32, tag="mm", name="ag")
                    for c in range(ND):
                        nc.tensor.matmul(
                            ps[:], wqT_s[:, c, ds(d * P, P)], xs_s[:, c, :],
                            start=(c == 0), stop=(c == ND - 1))
                    nc.scalar.activation(
                        asum_s[:, d, :], ps[:],
                        mybir.ActivationFunctionType.Identity,
                        bias=bq64_s[:, d:d + 1])
                # AWQ^T[c, j] and AWK^T[c, j], interleaved per batch as the
                # fused [128, 32] logit stationary
                for (w_s, half) in ((wqN_s, 0), (wkN_s, 2)):
                    for c in range(ND):
                        ps = ps_big.tile([P, B_LOC, M], F32, tag="mm", name="ag")
                        for d in range(ND):
                            nc.tensor.matmul(
                                ps[:], w_s[:, d, ds(c * P, P)], asum_s[:, d, :],
                                start=(d == 0), stop=(d == ND - 1))
                        nc.scalar.activation(
                            awqk_s[:, c, :, half * M:(half + 1) * M], ps[:],
                            mybir.ActivationFunctionType.Copy)

            def l12(b):
                # E12^T[0:16, n] = exp(s*Q A^T + c1) ; [16:32, n] = exp(s*A K^T)
                s = st[b]
                s["e12t"] = e12t = pe12.tile([3 * M, N], BF16, tag="e12t",
                                             name=f"e12t_{b}")
                d2 = psmall.tile([3 * M, 2], F32, tag="d2", name=f"d2_{b}")
                for ni in range(2):
                    lg = ps_log.tile([3 * M, 512], F32, tag="log", name=f"log_{b}")
                    for c in range(ND):
                        nc.tensor.matmul(
                            lg[:], awqk_s[:, c, b, :], s["xt"][:, c, ts(ni, 512)],
                            start=(c == 0), stop=(c == ND - 1))
                    nc.scalar.activation(
                        e12t[:, ts(ni, 512)], lg[:],
                        mybir.ActivationFunctionType.Exp,
                        bias=c12b_s[:, b:b + 1], scale=float(SCALE),
                        accum_out=d2[:, ni:ni + 1])
                s["d2"] = d2

            def r2c(b):
                s = st[b]
                d2 = s["d2"]
                d2s = psmall.tile([M, 1], F32, tag="d2s", name=f"d2s_{b}")
                nc.vector.tensor_add(d2s[:], d2[2 * M:3 * M, 0:1], d2[2 * M:3 * M, 1:2])
                s["r2"] = r2 = psmall.tile([M, 1], F32, tag="r2", name=f"r2_{b}")
                nc.vector.reciprocal(r2[:], d2s[:])

            def tr(b):
                # e12p[p, o, i] = E12^T[i, o*128+p] via XBAR dma transpose
                s = st[b]
                s["e12p"] = e12p = pe12p.tile([P, NN, 3 * M], BF16, tag="e12p",
                                              name=f"e12p_{b}")
                e12t = s["e12t"]
                h = NN // 2
                nc.sync.dma_start_transpose(e12p[:, 0:h, :], e12t[:, 0:512])
                nc.sync.dma_start_transpose(e12p[:, h:NN, :], e12t[:, 512:])

            def r1(b):
                # r1[n] = sum_m E1^T[m, n]; free-dim reduce in token-partition
                s = st[b]
                r_s = psmall.tile([P, NN], F32, tag="r_s", name=f"r_s_{b}")
                nc.vector.reduce_sum(r_s[:], s["e12p"][:, :, 0:M],
                                     axis=mybir.AxisListType.X)
                s["r_inv"] = r_inv = psmall.tile([P, NN], F32, tag="r_inv",
                                                 name=f"r_inv_{b}")
                nc.vector.reciprocal(r_inv[:], r_s[:])

            def ex(b):
                # exr = diag(r2) * (E2 @ x)   [16, C]
                s = st[b]
                pse = ps_se.tile([M, C], F32, tag="se", name=f"se_{b}")
                for n in range(NN):
                    nc.tensor.matmul(
                        pse[:], s["e12p"][:, n, 2 * M:3 * M], s["xn"][:, n, :],
                        start=(n == 0), stop=(n == NN - 1))
                s["exr"] = exr = psmall.tile([M, C], BF16, tag="exr",
                                             name=f"exr_{b}")
                nc.scalar.activation(
                    exr[:], pse[:], mybir.ActivationFunctionType.Copy,
                    scale=s["r2"][:])

            def exrT(b):
                # exrT[p, c, m] = exr[m, c*128+p] via PE transposes
                s = st[b]
                pst = ps_se.tile([P, ND, M], BF16, tag="se", name=f"tr_{b}")
                for c in range(ND):
                    nc.tensor.transpose(pst[:, c, :], s["exr"][:, ts(c, P)],
                                        ident[:])
                s["exrT"] = exrT_ = psmall.tile([P, ND, M], BF16, tag="exrT",
                                                name=f"exrT_{b}")
                nc.vector.tensor_copy(exrT_[:], pst[:])

            def afw(b):
                # afw = exr @ Wvo^T   [16, C]
                s = st[b]
                psa = ps_se.tile([M, C], F32, tag="se", name=f"afw_{b}")
                for c in range(ND):
                    nc.tensor.matmul(
                        psa[:], s["exrT"][:, c, :], wvo_s[:, c, :],
                        start=(c == 0), stop=(c == ND - 1))
                s["afw"] = afw_ = psmall.tile([M, C], BF16, tag="afw",
                                              name=f"afw_{b}")
                nc.scalar.activation(
                    afw_[:], psa[:], mybir.ActivationFunctionType.Copy)
                nc.vector.tensor_add(afw_[:], afw_[:], bp_s[0:M, :])

            def outp(b):
                # out chunk = r1 * (E1 @ afw) + b'
                s = st[b]
                s["o"] = o_s = pout.tile([P, NN, C], BF16, tag="o", name=f"o_{b}")
                for n in range(NN):
                    po = ps_big.tile([P, C], F32, tag="mm", name=f"mm_{b}")
                    nc.tensor.matmul(
                        po[:], s["e12t"][0:M, ts(n, P)], s["afw"][:],
                        start=True, stop=True)
                    if n % 2 == 0:
                        nc.scalar.activation(
                            o_s[:, n, :], po[:],
                            mybir.ActivationFunctionType.Copy,
                            scale=s["r_inv"][:, n:n + 1])
                    else:
                        nc.vector.tensor_scalar_mul(
                            o_s[:, n, :], po[:], s["r_inv"][:, n:n + 1])

            def store(b):
                nc.sync.dma_start(out_d.ap()[b], st[b]["o"][:])

            # prologue: agent products overlap the first x loads
            load_x(0)
            nc.sync.dma_start(wqN_s[:], wqN_d.ap())
            nc.sync.dma_start(wkN_s[:], wkN_d.ap())
            nc.sync.dma_start(c12b_s[:], c12b_d.ap())
            agent_stage()
            load_x(1)
            nc.sync.dma_start(wvo_s[:], wvo_d.ap())
            nc.sync.dma_start(bp_s[:], bp_d.ap())
            load_x(2)

            for b in range(B_LOC):
                l12(b)
                tr(b)
                if b + 3 < B_LOC:
                    load_x(b + 3)
                if b >= 1:
                    ex(b - 1)
                if b >= 2:
                    outp(b - 2)
                    store(b - 2)
                if b >= 1:
                    exrT(b - 1)
                    afw(b - 1)
                r2c(b)
                r1(b)
            L = B_LOC - 1
            ex(L)
            outp(L - 1)
            store(L - 1)
            exrT(L)
            afw(L)
            outp(L)
            store(L)

    nc.compile()
    return nc


def _prep_inputs(x, Wq, bq, Wk, bk, Wv, bv, Wo, bo):
    bf = ml_dtypes.bfloat16
    x32 = np.asarray(x, np.float32)
    # xt[b, p, o, n] = x[b, n, o*128+p]  (contiguous 8KB per partition)
    xt = np.ascontiguousarray(
        x32.transpose(0, 2, 1).reshape(B, ND, P, N).transpose(0, 2, 1, 3)
    ).astype(bf)
    # xn[b, p, o, c] = x[b, o*128+p, c]
    xn = np.ascontiguousarray(
        x32.reshape(B, NN, P, C).transpose(0, 2, 1, 3)).astype(bf)
    # pooled sums, all local batches stacked: xs[p, o, b*16+m]
    xsum = x32.reshape(B, M, G, C).sum(axis=2)  # [B, M, C]
    Wo64 = np.asarray(Wo, np.float64)
    Wv64 = np.asarray(Wv, np.float64)

    def wtile(w):  # [C, C] -> [P, ND, C] with w[p, o, d] = W[o*128+p, d]
        return np.ascontiguousarray(
            np.asarray(w, np.float32).reshape(ND, P, C).transpose(1, 0, 2)
        ).astype(bf)

    shared = {
        "wqT": wtile(np.asarray(Wq, np.float32).T),
        "wqN": wtile(np.asarray(Wq, np.float32)),
        "wkN": wtile(np.asarray(Wk, np.float32)),
        "wvoT": wtile((Wo64 @ Wv64).T.astype(np.float32)),
        "bq64": np.ascontiguousarray(
            (64.0 * np.asarray(bq, np.float32)).reshape(ND, P).T),
    }
    bprime = np.asarray(bo, np.float64) + Wo64 @ np.asarray(bv, np.float64)
    shared["bp"] = np.tile(bprime.astype(np.float32), (P, 1)).astype(bf)
    in_maps = []
    for ci in range(N_CORES):
        m = dict(shared)
        m["xt"] = np.ascontiguousarray(xt[ci * B_LOC:(ci + 1) * B_LOC])
        m["xn"] = np.ascontiguousarray(xn[ci * B_LOC:(ci + 1) * B_LOC])
        xs_c = xsum[ci * B_LOC:(ci + 1) * B_LOC]  # [B_LOC, M, C]
        xs_j = xs_c.transpose(2, 0, 1).reshape(C, J)  # [C, J], j = b*16+m
        m["xs"] = np.ascontiguousarray(
            xs_j.reshape(ND, P, J).transpose(1, 0, 2)).astype(bf)
        # stage-1 logit bias c1[b, m] = SCALE * (Asum_b,m . bq), exact on host
        asum64 = xs_c.astype(np.float64) @ np.asarray(Wq, np.float64).T \
            + 64.0 * np.asarray(bq, np.float64)
        c1 = SCALE * (asum64 @ np.asarray(bq, np.float64))  # [B_LOC, M]
        c12b = np.zeros((3 * M, B_LOC), np.float32)
        c12b[0:M, :] = c1.T.astype(np.float32)
        m["c12b"] = c12b
        in_maps.append(m)
    return in_maps


def _unpermute_out(res):
    # out_d[b, p, o, c] = out[b, o*128+p, c]
    outs = []
    for ci in range(N_CORES):
        o = np.asarray(res.results[ci]["out"], np.float32)  # [B_LOC, P, NN, C]
        outs.append(o.transpose(0, 2, 1, 3).reshape(B_LOC, N, C))
    return np.concatenate(outs, axis=0)


def kernel(x, Wq, bq, Wk, bk, Wv, bv, Wo, bo):
    global _LAST_RESULTS
    nc = _build_program()
    in_maps = _prep_inputs(x, Wq, bq, Wk, bk, Wv, bv, Wo, bo)
    res = run_bass_kernel_spmd(nc, in_maps, list(range(N_CORES)), **_RUN_KWARGS)
    _LAST_RESULTS = res
    return _unpermute_out(res)


# revision 9
# speedup vs baseline: 1.4329x; 1.1560x over previous
"""AgentAttention Trainium2 kernel (B=64, N=1024, C=512, M=16 agents) on 8 NeuronCores.

Data-parallel over batch: each core processes 8 batch elements. No collectives.

Math (per batch element, reference semantics):
    Q = x@Wq.T+bq ; K = x@Wk.T+bk ; V = x@Wv.T+bv
    A = group-mean of Q over 64-token groups          -> [16, C]
    S1 = softmax(Q A^T / sqrt(C), axis=agents)        -> [N, 16]
    S2 = softmax(A K^T / sqrt(C), axis=tokens)        -> [16, N]
    out = (S1 @ (S2 @ V)) @ Wo.T + bo

Algebraic restructuring (exact in real arithmetic):
    - bv never materialized: softmax rows sum to 1 =>  out += (Wo@bv + bo) == b'
    - A uses group-SUM; the 1/64 is folded into the logit scale s = 1/(64*sqrt(C))
    - re-association: S1@((S2@x)@(Wv^T Wo^T)) replaces the O(N*C^2) V- and
      output-projections with agent-space (M=16) ops; Wvo^T = (Wo@Wv)^T is
      precomputed on host in float64.
    - the A@bk logit shift is constant along the stage-2 softmax axis and
      cancels; it is dropped entirely.
    - stage-1 softmax normalizer r1 applied as a per-row scale on the final
      output tile; stage-2 normalizer r2 applied when evicting (S2@x) from PSUM.

Perf structure (v2):
    - agent products (Asum, AWQ^T, AWK^T, c1) computed ONCE for all 8 local
      batches with the 8*16=128 (batch,agent) pairs as a full 128-wide matmul
      free/partition dim -- 48 full-width matmuls instead of 448 16-wide ones.
    - AWQ^T/AWK^T stored interleaved as one [128, 32] stationary per
      (c-chunk, batch): stage-1 and stage-2 logits come out of ONE x^T stream.
    - per batch ONE pair of XBAR transposes moves [32, 1024] E1/E2 rows into
      token-partition layout (feeds both the ex matmul and the r1 reduce).
    - all HBM<->SBUF transfers are host-permuted so every SBUF partition
      reads/writes a single contiguous 8KB block (large DMA packets).
"""

import sys

if "/opt/trn_rl_repo" not in sys.path:
    sys.path.insert(0, "/opt/trn_rl_repo")

import numpy as np
import ml_dtypes

import concourse.bass as bass
import concourse.mybir as mybir
import concourse.tile as tile
from concourse import bacc
from concourse.bass import ts, ds
from concourse.bass_utils import run_bass_kernel_spmd
from concourse.masks import make_identity

BF16 = mybir.dt.bfloat16
F32 = mybir.dt.float32
F8 = mybir.dt.float8e4

N_CORES = 8
B = 64
B_LOC = B // N_CORES  # 8 batches per core
N = 1024              # tokens
C = 512               # channels
M = 16                # agents
G = N // M            # 64-token pooling groups
P = 128
ND = C // P           # 4 channel chunks
NN = N // P           # 8 token chunks of 128
J = B_LOC * M         # 128 stacked (batch, agent) columns
SCALE = 1.0 / (G * np.sqrt(C))  # logit scale (1/64 pooling fold included)

# test harness may override (e.g. {"trace": True, "tmpdir": ...})
_RUN_KWARGS = {}
_LAST_RESULTS = None


def _build_program():
    nc = bacc.Bacc("TRN2", target_bir_lowering=False, debug=False,
                   num_devices=N_CORES)

    xt_d = nc.dram_tensor("xt", [B_LOC, P, 2, 2, N], F8, kind="ExternalInput")
    xn_d = nc.dram_tensor("xn", [B_LOC, P, NN, C], BF16, kind="ExternalInput")
    xs_d = nc.dram_tensor("xs", [P, ND, J], BF16, kind="ExternalInput")
    wqT_d = nc.dram_tensor("wqT", [P, ND, C], BF16, kind="ExternalInput")
    wqN_d = nc.dram_tensor("wqN", [P, ND, C], BF16, kind="ExternalInput")
    wkN_d = nc.dram_tensor("wkN", [P, ND, C], BF16, kind="ExternalInput")
    wvo_d = nc.dram_tensor("wvoT", [P, ND, C], BF16, kind="ExternalInput")
    bq64_d = nc.dram_tensor("bq64", [P, ND], F32, kind="ExternalInput")
    bp_d = nc.dram_tensor("bp", [P, C], BF16, kind="ExternalInput")
    c12b_d = nc.dram_tensor("c12b", [3 * M, B_LOC], F32, kind="ExternalInput")
    out_d = nc.dram_tensor("out", [B_LOC, P, NN, C], BF16, kind="ExternalOutput")

    with tile.TileContext(nc) as tc:
        with (
            tc.tile_pool(name="const", bufs=1) as const,
            tc.tile_pool(name="pxt", bufs=4) as pxt,
            tc.tile_pool(name="pxn", bufs=4) as pxn,
            tc.tile_pool(name="pe12", bufs=3) as pe12,
            tc.tile_pool(name="pe12p", bufs=3) as pe12p,
            tc.tile_pool(name="psmall", bufs=4) as psmall,
            tc.tile_pool(name="pout", bufs=3) as pout,
            tc.tile_pool(name="ps_big", bufs=3, space="PSUM") as ps_big,
            tc.tile_pool(name="ps_log", bufs=2, space="PSUM") as ps_log,
            tc.tile_pool(name="ps_se", bufs=2, space="PSUM") as ps_se,
        ):
            wqT_s = const.tile([P, ND, C], BF16)
            wqN_s = const.tile([P, ND, C], BF16)
            wkN_s = const.tile([P, ND, C], BF16)
            wvo_s = const.tile([P, ND, C], BF16)
            xs_s = const.tile([P, ND, J], BF16)
            bq64_s = const.tile([P, ND], F32)
            bp_s = const.tile([P, C], BF16)
            asum_s = const.tile([P, ND, J], BF16)
            awqk_s = const.tile([P, 2, 2, B_LOC, 3 * M], F8)
            c12b_s = const.tile([3 * M, B_LOC], F32)
            ident = const.tile([M, M], BF16)

            # first wave of const loads: everything the agent stage needs
            nc.sync.dma_start(wqT_s[:], wqT_d.ap())
            nc.sync.dma_start(xs_s[:], xs_d.ap())
            nc.sync.dma_start(bq64_s[:], bq64_d.ap())
            make_identity(nc, ident)
            nc.vector.memset(awqk_s[:, :, :, :, M:2 * M], 0.0)

            st = [dict() for _ in range(B_LOC)]

            def load_x(b):
                s = st[b]
                s["xt"] = xt = pxt.tile([P, 2, 2, N], F8, tag="xt", name=f"xt_{b}")
                nc.sync.dma_start(xt[:], xt_d.ap()[b])
                s["xn"] = xn = pxn.tile([P, NN, C], BF16, tag="xn", name=f"xn_{b}")
                nc.sync.dma_start(xn[:], xn_d.ap()[b])

            def agent_stage():
                # AsumT[d, j] = sum_c Wq^T[c, d] xsum^T[c, j] + 64 bq[d]
                for d in range(ND):
                    ps = ps_big.tile([P, B_LOC, M], F32, tag="mm", name="ag")
                    for c in range(ND):
                        nc.tensor.matmul(
                            ps[:], wqT_s[:, c, ds(d * P, P)], xs_s[:, c, :],
                            start=(c == 0), stop=(c == ND - 1))
                    nc.scalar.activation(
                        asum_s[:, d, :], ps[:],
                        mybir.ActivationFunctionType.Identity,
                        bias=bq64_s[:, d:d + 1])
                # AWQ^T[c, j] and AWK^T[c, j], interleaved per batch as the
                # fused [128, 32] logit stationary
                for (w_s, half) in ((wqN_s, 0), (wkN_s, 2)):
                    for c in range(ND):
                        ps = ps_big.tile([P, B_LOC, M], F32, tag="mm", name="ag")
                        for d in range(ND):
                            nc.tensor.matmul(
                                ps[:], w_s[:, d, ds(c * P, P)], asum_s[:, d, :],
                                start=(d == 0), stop=(d == ND - 1))
                        nc.scalar.activation(
                            awqk_s[:, c // 2, c % 2, :,
                                   half * M:(half + 1) * M], ps[:],
                            mybir.ActivationFunctionType.Copy)

            def l12(b):
                # E12^T[0:16, n] = exp(s*Q A^T + c1) ; [16:32, n] = exp(s*A K^T)
                s = st[b]
                s["e12t"] = e12t = pe12.tile([3 * M, N], BF16, tag="e12t",
                                             name=f"e12t_{b}")
                d2 = psmall.tile([3 * M, 2], F32, tag="d2", name=f"d2_{b}")
                for ni in range(2):
                    lg = ps_log.tile([3 * M, 512], F32, tag="log", name=f"log_{b}")
                    for kk in range(2):
                        nc.tensor.matmul(
                            lg[:], awqk_s[:, kk, :, b, :],
                            s["xt"][:, kk, :, ts(ni, 512)],
                            start=(kk == 0), stop=(kk == 1),
                            perf_mode=mybir.MatmulPerfMode.DoubleRow)
                    nc.scalar.activation(
                        e12t[:, ts(ni, 512)], lg[:],
                        mybir.ActivationFunctionType.Exp,
                        bias=c12b_s[:, b:b + 1], scale=float(SCALE),
                        accum_out=d2[:, ni:ni + 1])
                s["d2"] = d2

            def r2c(b):
                s = st[b]
                d2 = s["d2"]
                d2s = psmall.tile([M, 1], F32, tag="d2s", name=f"d2s_{b}")
                nc.vector.tensor_add(d2s[:], d2[2 * M:3 * M, 0:1], d2[2 * M:3 * M, 1:2])
                s["r2"] = r2 = psmall.tile([M, 1], F32, tag="r2", name=f"r2_{b}")
                nc.vector.reciprocal(r2[:], d2s[:])

            def tr(b):
                # e12p[p, o, i] = E12^T[i, o*128+p] via XBAR dma transpose
                s = st[b]
                s["e12p"] = e12p = pe12p.tile([P, NN, 3 * M], BF16, tag="e12p",
                                              name=f"e12p_{b}")
                e12t = s["e12t"]
                h = NN // 2
                nc.sync.dma_start_transpose(e12p[:, 0:h, :], e12t[:, 0:512])
                nc.sync.dma_start_transpose(e12p[:, h:NN, :], e12t[:, 512:])

            def r1(b):
                # r1[n] = sum_m E1^T[m, n]; free-dim reduce in token-partition
                s = st[b]
                r_s = psmall.tile([P, NN], F32, tag="r_s", name=f"r_s_{b}")
                nc.vector.reduce_sum(r_s[:], s["e12p"][:, :, 0:M],
                                     axis=mybir.AxisListType.X)
                s["r_inv"] = r_inv = psmall.tile([P, NN], F32, tag="r_inv",
                                                 name=f"r_inv_{b}")
                nc.vector.reciprocal(r_inv[:], r_s[:])

            def ex(b):
                # exr = diag(r2) * (E2 @ x)   [16, C]
                s = st[b]
                pse = ps_se.tile([M, C], F32, tag="se", name=f"se_{b}")
                for n in range(NN):
                    nc.tensor.matmul(
                        pse[:], s["e12p"][:, n, 2 * M:3 * M], s["xn"][:, n, :],
                        start=(n == 0), stop=(n == NN - 1))
                s["exr"] = exr = psmall.tile([M, C], BF16, tag="exr",
                                             name=f"exr_{b}")
                nc.scalar.activation(
                    exr[:], pse[:], mybir.ActivationFunctionType.Copy,
                    scale=s["r2"][:])

            def exrT(b):
                # exrT[p, c, m] = exr[m, c*128+p] via PE transposes
                s = st[b]
                pst = ps_se.tile([P, ND, M], BF16, tag="se", name=f"tr_{b}")
                for c in range(ND):
                    nc.tensor.transpose(pst[:, c, :], s["exr"][:, ts(c, P)],
                                        ident[:])
                s["exrT"] = exrT_ = psmall.tile([P, ND, M], BF16, tag="exrT",
                                                name=f"exrT_{b}")
                nc.vector.tensor_copy(exrT_[:], pst[:])

            def afw(b):
                # afw = exr @ Wvo^T   [16, C]
                s = st[b]
                psa = ps_se.tile([M, C], F32, tag="se", name=f"afw_{b}")
                for c in range(ND):
                    nc.tensor.matmul(
                        psa[:], s["exrT"][:, c, :], wvo_s[:, c, :],
                        start=(c == 0), stop=(c == ND - 1))
                s["afw"] = afw_ = psmall.tile([M, C], BF16, tag="afw",
                                              name=f"afw_{b}")
                nc.scalar.activation(
                    afw_[:], psa[:], mybir.ActivationFunctionType.Copy)
                nc.vector.tensor_add(afw_[:], afw_[:], bp_s[0:M, :])

            def outp(b):
                # out chunk = r1 * (E1 @ afw) + b'
                s = st[b]
                s["o"] = o_s = pout.tile([P, NN, C], BF16, tag="o", name=f"o_{b}")
                for n in range(NN):
                    po = ps_big.tile([P, C], F32, tag="mm", name=f"mm_{b}")
                    nc.tensor.matmul(
                        po[:], s["e12t"][0:M, ts(n, P)], s["afw"][:],
                        start=True, stop=True)
                    if n % 2 == 0:
                        nc.scalar.activation(
                            o_s[:, n, :], po[:],
                            mybir.ActivationFunctionType.Copy,
                            scale=s["r_inv"][:, n:n + 1])
                    else:
                        nc.vector.tensor_scalar_mul(
                            o_s[:, n, :], po[:], s["r_inv"][:, n:n + 1])

            def store(b):
                nc.sync.dma_start(out_d.ap()[b], st[b]["o"][:])

            # prologue: agent products overlap the first x loads
            load_x(0)
            nc.sync.dma_start(wqN_s[:], wqN_d.ap())
            nc.sync.dma_start(wkN_s[:], wkN_d.ap())
            nc.sync.dma_start(c12b_s[:], c12b_d.ap())
            agent_stage()
            load_x(1)
            nc.sync.dma_start(wvo_s[:], wvo_d.ap())
            nc.sync.dma_start(bp_s[:], bp_d.ap())
            load_x(2)

            for b in range(B_LOC):
                l12(b)
                tr(b)
                if b + 3 < B_LOC:
                    load_x(b + 3)
                if b >= 1:
                    ex(b - 1)
                if b >= 2:
                    outp(b - 2)
                    store(b - 2)
                if b >= 1:
                    exrT(b - 1)
                    afw(b - 1)
                r2c(b)
                r1(b)
            L = B_LOC - 1
            ex(L)
            outp(L - 1)
            store(L - 1)
            exrT(L)
            afw(L)
            outp(L)
            store(L)

    nc.compile()
    return nc


def _prep_inputs(x, Wq, bq, Wk, bk, Wv, bv, Wo, bo):
    bf = ml_dtypes.bfloat16
    x32 = np.asarray(x, np.float32)
    # xt[b, p, kk, i, n] = x[b, n, kk*256+i*128+p]  (fp8 DoubleRow planes)
    f8 = ml_dtypes.float8_e4m3
    xt = np.ascontiguousarray(
        x32.transpose(0, 2, 1).reshape(B, 2, 2, P, N).transpose(0, 3, 1, 2, 4)
    ).astype(f8)
    # xn[b, p, o, c] = x[b, o*128+p, c]
    xn = np.ascontiguousarray(
        x32.reshape(B, NN, P, C).transpose(0, 2, 1, 3)).astype(bf)
    # pooled sums, all local batches stacked: xs[p, o, b*16+m]
    xsum = x32.reshape(B, M, G, C).sum(axis=2)  # [B, M, C]
    Wo64 = np.asarray(Wo, np.float64)
    Wv64 = np.asarray(Wv, np.float64)

    def wtile(w):  # [C, C] -> [P, ND, C] with w[p, o, d] = W[o*128+p, d]
        return np.ascontiguousarray(
            np.asarray(w, np.float32).reshape(ND, P, C).transpose(1, 0, 2)
        ).astype(bf)

    shared = {
        "wqT": wtile(np.asarray(Wq, np.float32).T),
        "wqN": wtile(np.asarray(Wq, np.float32)),
        "wkN": wtile(np.asarray(Wk, np.float32)),
        "wvoT": wtile((Wo64 @ Wv64).T.astype(np.float32)),
        "bq64": np.ascontiguousarray(
            (64.0 * np.asarray(bq, np.float32)).reshape(ND, P).T),
    }
    bprime = np.asarray(bo, np.float64) + Wo64 @ np.asarray(bv, np.float64)
    shared["bp"] = np.tile(bprime.astype(np.float32), (P, 1)).astype(bf)
    in_maps = []
    for ci in range(N_CORES):
        m = dict(shared)
        m["xt"] = np.ascontiguousarray(xt[ci * B_LOC:(ci + 1) * B_LOC])
        m["xn"] = np.ascontiguousarray(xn[ci * B_LOC:(ci + 1) * B_LOC])
        xs_c = xsum[ci * B_LOC:(ci + 1) * B_LOC]  # [B_LOC, M, C]
        xs_j = xs_c.transpose(2, 0, 1).reshape(C, J)  # [C, J], j = b*16+m
        m["xs"] = np.ascontiguousarray(
            xs_j.reshape(ND, P, J).transpose(1, 0, 2)).astype(bf)
        # stage-1 logit bias c1[b, m] = SCALE * (Asum_b,m . bq), exact on host
        asum64 = xs_c.astype(np.float64) @ np.asarray(Wq, np.float64).T \
            + 64.0 * np.asarray(bq, np.float64)
        c1 = SCALE * (asum64 @ np.asarray(bq, np.float64))  # [B_LOC, M]
        c12b = np.zeros((3 * M, B_LOC), np.float32)
        c12b[0:M, :] = c1.T.astype(np.float32)
        m["c12b"] = c12b
        in_maps.append(m)
    return in_maps


def _unpermute_out(res):
    # out_d[b, p, o, c] = out[b, o*128+p, c]
    outs = []
    for ci in range(N_CORES):
        o = np.asarray(res.results[ci]["out"], np.float32)  # [B_LOC, P, NN, C]
        outs.append(o.transpose(0, 2, 1, 3).reshape(B_LOC, N, C))
    return np.concatenate(outs, axis=0)


def kernel(x, Wq, bq, Wk, bk, Wv, bv, Wo, bo):
    global _LAST_RESULTS
    nc = _build_program()
    in_maps = _prep_inputs(x, Wq, bq, Wk, bk, Wv, bv, Wo, bo)
    res = run_bass_kernel_spmd(nc, in_maps, list(range(N_CORES)), **_RUN_KWARGS)
    _LAST_RESULTS = res
    return _unpermute_out(res)
